# Initial kernel scaffold
#
"""Multi-head attention (dense transformer block) on 8 Trainium2 NeuronCores.

Sharding: one attention head per core (H=8 heads, 8 cores), both batch
elements on every core; QKV/O weights are sliced per head on the host and
each core computes its head's full attention plus its partial contribution
to the output projection. Host sums the 8 partial projections (the only
cross-core reduction; there is no device-to-device communication).

Per-core pipeline (all matmul inputs fp16, fp32 accumulation):
  q/k/vT[e,n] = W_head @ X_b^T                 (PE, contraction d=512)
  S^T[j,i]    = (K^T chunk).T-matmul(Q^T)      (PE, contraction hd=64)
  P^T         = exp(S^T/8 - SHIFT)             (ACT; SHIFT exact for softmax)
  Ou^T[d,i]   = V_aug.T @ P^T                  (PE; V_aug has a ones column
                                                so row 64 accumulates l=sum P)
  part[i,e]   = (Ou_aug chunk).T @ w_o_aug     (PE; w_o_aug row 64 = b_o so
                                                the bias enters as l*b_o)
  out         = part * (1/l)                   (DVE at evacuation)
"""
import numpy as np
from contextlib import ExitStack

import concourse.bass as bass
import concourse.tile as tile
from concourse import bacc, mybir
from concourse.bass_utils import run_bass_kernel_spmd
from concourse.masks import make_identity

dt = mybir.dt

H = 8
HD = 64
D = 512
B = 2
N = 4096
SCALE = 0.125
SHIFT = 2.0  # global logit shift; exact for softmax, keeps exp() in fp16 range

MM_DT = dt.float16
MM_NP = np.float16


def _build(repeat=1, ichunk=1024, ppool_bufs=34, spool_bufs=2, xt_bufs=12,
           exp_width=1024):
    # ichunk=1024 measured 552us/iter vs 634 at ichunk=512 (interleaved A/B,
    # same 6.603e-4 rel err): halves ACT exp-op count and i-chunk pass
    # overhead. PSUM: spool 2x[128,1024](4 banks) + opool 2 + projpool 2 = 8.
    NB = B * N
    njb = N // 128
    nic = N // ichunk
    mm_per_ic = ichunk // 512
    nexp = ichunk // exp_width

    nc = bacc.Bacc("TRN2", target_bir_lowering=False, debug=False, num_devices=8)
    xt = nc.dram_tensor("xt", [D, NB], MM_DT, kind="ExternalInput").ap()
    wqkvt = nc.dram_tensor("wqkvt", [D, 3 * HD], MM_DT, kind="ExternalInput").ap()
    woat = nc.dram_tensor("woat", [HD + 1, D], MM_DT, kind="ExternalInput").ap()
    part = nc.dram_tensor("part", [B, N, D], dt.float32, kind="ExternalOutput").ap()

    with tile.TileContext(nc) as tc:
        with ExitStack() as ctx:
            const_p = ctx.enter_context(tc.tile_pool(name="const", bufs=1))
            xt_p = ctx.enter_context(tc.tile_pool(name="xt", bufs=xt_bufs))
            qkv_p = ctx.enter_context(tc.tile_pool(name="qkv", bufs=1))
            vaug_p = ctx.enter_context(tc.tile_pool(name="vaug", bufs=1))
            pt_p = ctx.enter_context(tc.tile_pool(name="pt", bufs=ppool_bufs))
            out_p = ctx.enter_context(tc.tile_pool(name="outs", bufs=6))
            ot_p = ctx.enter_context(tc.tile_pool(name="ot", bufs=4))
            small_p = ctx.enter_context(tc.tile_pool(name="small", bufs=8))
            spool = ctx.enter_context(tc.tile_pool(name="spool", bufs=spool_bufs, space="PSUM"))
            opool = ctx.enter_context(tc.tile_pool(name="opool", bufs=2, space="PSUM"))
            projpool = ctx.enter_context(tc.tile_pool(name="projpool", bufs=2, space="PSUM"))

            ident = const_p.tile([128, 128], MM_DT, tag="ident")
            make_identity(nc, ident[:])
            shiftc = const_p.tile([128, 1], dt.float32, tag="shiftc")
            nc.vector.memset(shiftc[:], -SHIFT)
            wq = const_p.tile([128, 4, 3 * HD], MM_DT, tag="wq")
            for d in range(4):
                nc.sync.dma_start(wq[:, d, :], wqkvt[d * 128:(d + 1) * 128, :])
            woa = const_p.tile([HD + 1, D], MM_DT, tag="woa")
            nc.sync.dma_start(woa[:], woat[:])

            qT = qkv_p.tile([64, NB], MM_DT, tag="qT")
            kT = qkv_p.tile([64, NB], MM_DT, tag="kT")
            vT = qkv_p.tile([64, NB], MM_DT, tag="vT")
            vaug = [vaug_p.tile([128, njb * 65], MM_DT, tag=f"vaug{b}", name=f"vaug{b}")
                    for b in range(B)]

            def body(_=None):
                # ---- QKV projection ----
                for b in range(B):
                    for ch in range(N // 512):
                        c0 = b * N + ch * 512
                        xts = []
                        for d in range(4):
                            t = xt_p.tile([128, 512], MM_DT, tag="xt", name="xt_t")
                            nc.sync.dma_start(t[:], xt[d * 128:(d + 1) * 128, c0:c0 + 512])
                            xts.append(t)
                        ps_q = spool.tile([64, 512], dt.float32, tag="s", name="ps_q")
                        ps_k = spool.tile([64, 512], dt.float32, tag="s", name="ps_k")
                        ps_v = spool.tile([64, 512], dt.float32, tag="s", name="ps_v")
                        for d in range(4):
                            nc.tensor.matmul(ps_q[:], wq[:, d, 0:HD], xts[d][:],
                                             start=(d == 0), stop=(d == 3))
                        for d in range(4):
                            nc.tensor.matmul(ps_k[:], wq[:, d, HD:2 * HD], xts[d][:],
                                             start=(d == 0), stop=(d == 3))
                        for d in range(4):
                            nc.tensor.matmul(ps_v[:], wq[:, d, 2 * HD:3 * HD], xts[d][:],
                                             start=(d == 0), stop=(d == 3))
                        nc.vector.tensor_copy(qT[:, c0:c0 + 512], ps_q[:])
                        nc.vector.tensor_copy(kT[:, c0:c0 + 512], ps_k[:])
                        nc.vector.tensor_copy(vT[:, c0:c0 + 512], ps_v[:])

                # ---- V_aug (V transposed + ones column) ----
                for b in range(B):
                    nc.vector.memset(vaug[b][:], 1.0)
                    for jb in range(njb):
                        pt = opool.tile([128, 64], MM_DT, tag="o", name="pt_tr")
                        nc.tensor.transpose(
                            pt[:], vT[:, b * N + jb * 128: b * N + (jb + 1) * 128],
                            ident[0:64, 0:64])
                        nc.vector.tensor_copy(vaug[b][:, jb * 65: jb * 65 + 64], pt[:])

                # ---- attention + projection ----
                for b in range(B):
                    for ic in range(nic):
                        i0 = b * N + ic * ichunk
                        pts = []
                        for jb in range(njb):
                            ps_s = spool.tile([128, ichunk], dt.float32, tag="s", name="ps_s")
                            for m in range(mm_per_ic):
                                nc.tensor.matmul(
                                    ps_s[:, m * 512:(m + 1) * 512],
                                    kT[:, b * N + jb * 128: b * N + (jb + 1) * 128],
                                    qT[:, i0 + m * 512: i0 + (m + 1) * 512],
                                    start=True, stop=True)
                            ptile = pt_p.tile([128, ichunk], MM_DT, tag="pt", name="ptile")
                            for e in range(nexp):
                                nc.scalar.activation(
                                    ptile[:, e * exp_width:(e + 1) * exp_width],
                                    ps_s[:, e * exp_width:(e + 1) * exp_width],
                                    mybir.ActivationFunctionType.Exp,
                                    bias=shiftc[:, 0:1], scale=SCALE)
                            pts.append(ptile)
                        ps_o = [opool.tile([65, 512], dt.float32, tag="o", name="ps_o")
                                for _ in range(mm_per_ic)]
                        for jb in range(njb):
                            for m in range(mm_per_ic):
                                nc.tensor.matmul(
                                    ps_o[m][:],
                                    vaug[b][:, jb * 65:(jb + 1) * 65],
                                    pts[jb][:, m * 512:(m + 1) * 512],
                                    start=(jb == 0), stop=(jb == njb - 1))
                        for m in range(mm_per_ic):
                            ouT = ot_p.tile([65, 512], MM_DT, tag="ot", name="ouT")
                            nc.vector.tensor_copy(ouT[:], ps_o[m][:])
                            lrec = small_p.tile([65, 512], dt.float32, tag="lrec", name="lrec")
                            nc.vector.reciprocal(lrec[64:65, :], ps_o[m][64:65, :])
                            lrecT = small_p.tile([128, 4], dt.float32, tag="lrecT", name="lrecT")
                            for ib in range(4):
                                nc.sync.dma_start(
                                    lrecT[:, ib:ib + 1],
                                    lrec[64:65, ib * 128:(ib + 1) * 128])
                            for ib in range(4):
                                ps_p = projpool.tile([128, 512], dt.float32, tag="pj", name="ps_p")
                                nc.tensor.matmul(ps_p[:], ouT[:, ib * 128:(ib + 1) * 128],
                                                 woa[:], start=True, stop=True)
                                osb = out_p.tile([128, 512], dt.float32, tag="ou", name="osb")
                                nc.vector.tensor_scalar_mul(osb[:], ps_p[:], lrecT[:, ib:ib + 1])
                                row0 = ic * ichunk + m * 512 + ib * 128
                                nc.sync.dma_start(part[b, row0:row0 + 128, :], osb[:])

            if repeat == 1:
                body()
            else:
                with tc.For_i(0, repeat, 1) as _i:
                    body()

    nc.compile()
    return nc


def _make_in_maps(x, w_qkv, w_o, b_o):
    xt = np.ascontiguousarray(x.transpose(2, 1, 0).reshape(D, B * N)).astype(MM_NP)
    in_maps = []
    for c in range(8):
        wqs = w_qkv[c * HD:(c + 1) * HD]
        wks = w_qkv[D + c * HD:D + (c + 1) * HD]
        wvs = w_qkv[2 * D + c * HD:2 * D + (c + 1) * HD]
        wqkvt = np.ascontiguousarray(np.concatenate([wqs, wks, wvs], 0).T).astype(MM_NP)
        bo_row = b_o if c == 0 else np.zeros_like(b_o)
        woat = np.concatenate(
            [w_o[:, c * HD:(c + 1) * HD].T, bo_row[None, :]], 0).astype(MM_NP)
        in_maps.append({"xt": xt, "wqkvt": wqkvt, "woat": woat})
    return in_maps


_NC_CACHE = {}


def _get_nc(repeat=1, **kw):
    key = (repeat, tuple(sorted(kw.items())))
    if key not in _NC_CACHE:
        _NC_CACHE[key] = _build(repeat=repeat, **kw)
    return _NC_CACHE[key]


def kernel(x, w_qkv, w_o, b_o):
    x = np.asarray(x, np.float32)
    w_qkv = np.asarray(w_qkv, np.float32)
    w_o = np.asarray(w_o, np.float32)
    b_o = np.asarray(b_o, np.float32)
    assert x.shape == (N, B, D), x.shape
    nc = _get_nc()
    in_maps = _make_in_maps(x, w_qkv, w_o, b_o)
    res = run_bass_kernel_spmd(nc, in_maps, list(range(8)))
    acc = np.zeros((B, N, D), np.float64)
    for r in res.results:
        acc += r["part"]
    return acc.astype(np.float32)



# revision 1
# speedup vs baseline: 2.0214x; 2.0214x over previous
"""Multi-head attention (dense transformer block) on 8 Trainium2 NeuronCores.

Sharding: one attention head per core (H=8 heads, 8 cores), both batch
elements on every core; QKV/O weights are sliced per head on the host and
each core computes its head's full attention plus its partial contribution
to the output projection. Host sums the 8 partial projections (the only
cross-core reduction; there is no device-to-device communication).

Per-core pipeline (all matmul inputs fp16, fp32 accumulation):
  q/k/vT[e,n] = W_head @ X_b^T                 (PE, contraction d=512)
  S^T[j,i]    = (K^T chunk).T-matmul(Q^T)      (PE, contraction hd=64)
  P^T         = exp(S^T/8 - SHIFT)             (ACT; SHIFT exact for softmax)
  Ou^T[d,i]   = V_aug.T @ P^T                  (PE; V_aug has a ones column
                                                so row 64 accumulates l=sum P)
  part[i,e]   = (Ou_aug chunk).T @ w_o_aug     (PE; w_o_aug row 64 = b_o so
                                                the bias enters as l*b_o)
  out         = part * (1/l)                   (DVE at evacuation)
"""
import numpy as np
from contextlib import ExitStack

import concourse.bass as bass
import concourse.tile as tile
from concourse import bacc, mybir
from concourse.bass_utils import run_bass_kernel_spmd
from concourse.masks import make_identity

dt = mybir.dt

H = 8
HD = 64
D = 512
B = 2
N = 4096
SCALE = 0.125
SHIFT = 2.0  # global logit shift; exact for softmax, keeps exp() in fp16 range

MM_DT = dt.float16
MM_NP = np.float16


def _build(repeat=1, ichunk=1024, ppool_bufs=34, spool_bufs=2, xt_bufs=12,
           exp_width=1024):
    # ichunk=1024 measured 552us/iter vs 634 at ichunk=512 (interleaved A/B,
    # same 6.603e-4 rel err): halves ACT exp-op count and i-chunk pass
    # overhead. PSUM: spool 2x[128,1024](4 banks) + opool 2 + projpool 2 = 8.
    NB = B * N
    njb = N // 128
    nic = N // ichunk
    mm_per_ic = ichunk // 512
    nexp = ichunk // exp_width

    nc = bacc.Bacc("TRN2", target_bir_lowering=False, debug=False, num_devices=8)
    xt = nc.dram_tensor("xt", [D, NB], MM_DT, kind="ExternalInput").ap()
    wqkvt = nc.dram_tensor("wqkvt", [D, 3 * HD], MM_DT, kind="ExternalInput").ap()
    woat = nc.dram_tensor("woat", [HD + 1, D], MM_DT, kind="ExternalInput").ap()
    part = nc.dram_tensor("part", [B, N, D], dt.float32, kind="ExternalOutput").ap()

    with tile.TileContext(nc) as tc:
        with ExitStack() as ctx:
            const_p = ctx.enter_context(tc.tile_pool(name="const", bufs=1))
            xt_p = ctx.enter_context(tc.tile_pool(name="xt", bufs=xt_bufs))
            qkv_p = ctx.enter_context(tc.tile_pool(name="qkv", bufs=1))
            vaug_p = ctx.enter_context(tc.tile_pool(name="vaug", bufs=1))
            pt_p = ctx.enter_context(tc.tile_pool(name="pt", bufs=ppool_bufs))
            out_p = ctx.enter_context(tc.tile_pool(name="outs", bufs=6))
            ot_p = ctx.enter_context(tc.tile_pool(name="ot", bufs=4))
            small_p = ctx.enter_context(tc.tile_pool(name="small", bufs=8))
            spool = ctx.enter_context(tc.tile_pool(name="spool", bufs=spool_bufs, space="PSUM"))
            opool = ctx.enter_context(tc.tile_pool(name="opool", bufs=2, space="PSUM"))
            projpool = ctx.enter_context(tc.tile_pool(name="projpool", bufs=2, space="PSUM"))

            ident = const_p.tile([128, 128], MM_DT, tag="ident")
            make_identity(nc, ident[:])
            shiftc = const_p.tile([128, 1], dt.float32, tag="shiftc")
            nc.vector.memset(shiftc[:], -SHIFT)
            wq = const_p.tile([128, 4, 3 * HD], MM_DT, tag="wq")
            for d in range(4):
                nc.sync.dma_start(wq[:, d, :], wqkvt[d * 128:(d + 1) * 128, :])
            woa = const_p.tile([HD + 1, D], MM_DT, tag="woa")
            nc.sync.dma_start(woa[:], woat[:])

            qT = qkv_p.tile([64, NB], MM_DT, tag="qT")
            kT = qkv_p.tile([64, NB], MM_DT, tag="kT")
            vT = qkv_p.tile([64, NB], MM_DT, tag="vT")
            vaug = [vaug_p.tile([128, njb * 65], MM_DT, tag=f"vaug{b}", name=f"vaug{b}")
                    for b in range(B)]

            def body(_=None):
                # ---- QKV projection ----
                for b in range(B):
                    for ch in range(N // 512):
                        c0 = b * N + ch * 512
                        xts = []
                        for d in range(4):
                            t = xt_p.tile([128, 512], MM_DT, tag="xt", name="xt_t")
                            nc.sync.dma_start(t[:], xt[d * 128:(d + 1) * 128, c0:c0 + 512])
                            xts.append(t)
                        ps_q = spool.tile([64, 512], dt.float32, tag="s", name="ps_q")
                        ps_k = spool.tile([64, 512], dt.float32, tag="s", name="ps_k")
                        ps_v = spool.tile([64, 512], dt.float32, tag="s", name="ps_v")
                        for d in range(4):
                            nc.tensor.matmul(ps_q[:], wq[:, d, 0:HD], xts[d][:],
                                             start=(d == 0), stop=(d == 3))
                        for d in range(4):
                            nc.tensor.matmul(ps_k[:], wq[:, d, HD:2 * HD], xts[d][:],
                                             start=(d == 0), stop=(d == 3))
                        for d in range(4):
                            nc.tensor.matmul(ps_v[:], wq[:, d, 2 * HD:3 * HD], xts[d][:],
                                             start=(d == 0), stop=(d == 3))
                        nc.vector.tensor_copy(qT[:, c0:c0 + 512], ps_q[:])
                        nc.vector.tensor_copy(kT[:, c0:c0 + 512], ps_k[:])
                        nc.vector.tensor_copy(vT[:, c0:c0 + 512], ps_v[:])

                # ---- V_aug (V transposed + ones column) ----
                for b in range(B):
                    nc.vector.memset(vaug[b][:], 1.0)
                    for jb in range(njb):
                        pt = opool.tile([128, 64], MM_DT, tag="o", name="pt_tr")
                        nc.tensor.transpose(
                            pt[:], vT[:, b * N + jb * 128: b * N + (jb + 1) * 128],
                            ident[0:64, 0:64])
                        nc.vector.tensor_copy(vaug[b][:, jb * 65: jb * 65 + 64], pt[:])

                # ---- attention + projection ----
                for b in range(B):
                    for ic in range(nic):
                        i0 = b * N + ic * ichunk
                        pts = []
                        for jb in range(njb):
                            ps_s = spool.tile([128, ichunk], dt.float32, tag="s", name="ps_s")
                            for m in range(mm_per_ic):
                                nc.tensor.matmul(
                                    ps_s[:, m * 512:(m + 1) * 512],
                                    kT[:, b * N + jb * 128: b * N + (jb + 1) * 128],
                                    qT[:, i0 + m * 512: i0 + (m + 1) * 512],
                                    start=True, stop=True)
                            ptile = pt_p.tile([128, ichunk], MM_DT, tag="pt", name="ptile")
                            for e in range(nexp):
                                nc.scalar.activation(
                                    ptile[:, e * exp_width:(e + 1) * exp_width],
                                    ps_s[:, e * exp_width:(e + 1) * exp_width],
                                    mybir.ActivationFunctionType.Exp,
                                    bias=shiftc[:, 0:1], scale=SCALE)
                            pts.append(ptile)
                        ps_o = [opool.tile([65, 512], dt.float32, tag="o", name="ps_o")
                                for _ in range(mm_per_ic)]
                        for jb in range(njb):
                            for m in range(mm_per_ic):
                                nc.tensor.matmul(
                                    ps_o[m][:],
                                    vaug[b][:, jb * 65:(jb + 1) * 65],
                                    pts[jb][:, m * 512:(m + 1) * 512],
                                    start=(jb == 0), stop=(jb == njb - 1))
                        for m in range(mm_per_ic):
                            ouT = ot_p.tile([65, 512], MM_DT, tag="ot", name="ouT")
                            nc.vector.tensor_copy(ouT[:], ps_o[m][:])
                            lrec = small_p.tile([65, 512], dt.float32, tag="lrec", name="lrec")
                            nc.vector.reciprocal(lrec[64:65, :], ps_o[m][64:65, :])
                            lrecT = small_p.tile([128, 4], dt.float32, tag="lrecT", name="lrecT")
                            for ib in range(4):
                                nc.sync.dma_start(
                                    lrecT[:, ib:ib + 1],
                                    lrec[64:65, ib * 128:(ib + 1) * 128])
                            for ib in range(4):
                                ps_p = projpool.tile([128, 512], dt.float32, tag="pj", name="ps_p")
                                nc.tensor.matmul(ps_p[:], ouT[:, ib * 128:(ib + 1) * 128],
                                                 woa[:], start=True, stop=True)
                                osb = out_p.tile([128, 512], dt.float32, tag="ou", name="osb")
                                nc.vector.tensor_scalar_mul(osb[:], ps_p[:], lrecT[:, ib:ib + 1])
                                row0 = ic * ichunk + m * 512 + ib * 128
                                nc.sync.dma_start(part[b, row0:row0 + 128, :], osb[:])

            if repeat == 1:
                body()
            else:
                with tc.For_i(0, repeat, 1) as _i:
                    body()

    nc.compile()
    return nc


def _make_in_maps(x, w_qkv, w_o, b_o):
    xt = np.ascontiguousarray(x.transpose(2, 1, 0).reshape(D, B * N)).astype(MM_NP)
    in_maps = []
    for c in range(8):
        wqs = w_qkv[c * HD:(c + 1) * HD]
        wks = w_qkv[D + c * HD:D + (c + 1) * HD]
        wvs = w_qkv[2 * D + c * HD:2 * D + (c + 1) * HD]
        wqkvt = np.ascontiguousarray(np.concatenate([wqs, wks, wvs], 0).T).astype(MM_NP)
        bo_row = b_o if c == 0 else np.zeros_like(b_o)
        woat = np.concatenate(
            [w_o[:, c * HD:(c + 1) * HD].T, bo_row[None, :]], 0).astype(MM_NP)
        in_maps.append({"xt": xt, "wqkvt": wqkvt, "woat": woat})
    return in_maps


_NC_CACHE = {}


def _get_nc(repeat=1, **kw):
    key = (repeat, tuple(sorted(kw.items())))
    if key not in _NC_CACHE:
        _NC_CACHE[key] = _build(repeat=repeat, **kw)
    return _NC_CACHE[key]


def kernel(x, w_qkv, w_o, b_o):
    x = np.asarray(x, np.float32)
    w_qkv = np.asarray(w_qkv, np.float32)
    w_o = np.asarray(w_o, np.float32)
    b_o = np.asarray(b_o, np.float32)
    assert x.shape == (N, B, D), x.shape
    nc = _get_nc()
    in_maps = _make_in_maps(x, w_qkv, w_o, b_o)
    res = run_bass_kernel_spmd(nc, in_maps, list(range(8)))
    acc = np.zeros((B, N, D), np.float64)
    for r in res.results:
        acc += r["part"]
    return acc.astype(np.float32)

